# revision 25
# baseline (speedup 1.0000x reference)
"""Trainium2 Bass kernel for BPRLossWithNoClick.

Reference math (per sample b, L = x_lens[b], S = 1):
    loss_b = (1/L^2) * sum_{i<L, j<L} softplus(out[b,i,neg_ids[b,j,0]] - out[b,i,labels[b,j]])
    loss   = sum_b loss_b        (shape (1,), float32)

Strategy (8 NeuronCores, SPMD, all per-core variation carried in the data):
  * Only rows i < L_b of `output` are ever needed.  All valid rows across the
    batch are cut into 16-row "slots" and packed (host side) into per-core
    region tensors X[c] of shape [U, 128, V+2]: one region = up to 128 rows =
    8 slots, freely mixing samples (the 16-row slot granularity matches the
    per-16-partition index groups of the GPSIMD ap_gather instruction).
    The last region holds only p_last rows (p_last % 16 == 0) so the DMA
    reads almost exactly the valid bytes.
  * Rows are packed as float16 (the loss tolerates the quantization: the
    final error stays ~1e-6 relative).  ap_gather needs 4-byte granularity,
    so the kernel gathers uint32 *pairs* of f16 columns and selects the
    correct half per j with a host-provided parity predicate.
  * Each row carries a sentinel column pair (+big, -big): padded j slots
    gather pos=+big / neg=-big so softplus(neg-pos) underflows to exactly 0,
    removing the need for a j-validity mask.  Row validity and the 1/L^2
    scale live in a per-partition scalar fused into the final reduction.
  * Full regions are processed two-at-a-time per ap_gather (both halves of a
    [128, 2*(V+2)] SBUF tile; num_elems 2*(V+2)/2 <= 2^15) because each
    GPSIMD extended instruction has a ~12us engine-occupancy cost: halving
    the instruction count keeps the gather pipeline ahead of the DMA stream
    and shortens the kernel tail.
  * Device, per chunk (1-2 regions): DMA [p, n*(V+2)] f16 rows -> SBUF,
    ap_gather n*416 column pairs per 16-row group, parity-select, DVE
    subtract, softplus = Ln(Exp(d)+1) on ACT (both resolved to the one
    activation table that holds Exp AND Ln), per-partition scale with fused
    per-region reduction.  Output per core: [128, U] partial sums; host
    adds them up.

The kernel is DMA-bound (~32-40 MB of rows per core), which is the memory
roofline for this problem.
"""

import math

import numpy as np

_NCORES = 8
_P = 128           # partitions per full region
_SLOT = 16         # rows per slot == ap_gather index-group granularity
_GROUPS = _P // _SLOT
_JP = 208          # padded j capacity per slot (>= T=200, multiple of 16)
_NIDX = 2 * _JP    # gathered columns per region row (pos block + neg block)
_IDXW = _NIDX // 16  # int16 index words per partition per region
_SENT = 60000.0    # sentinel magnitude; softplus(-2*_SENT) == 0 exactly

_nc_cache = {}


def _chunks(U, p_last):
    """Group regions into gather chunks: full regions in pairs, the partial
    last region (if any) alone."""
    fulls = list(range(U if p_last == _P else U - 1))
    out = [fulls[i : i + 2] for i in range(0, len(fulls), 2)]
    if p_last != _P:
        out.append([U - 1])
    return out


def _prefer_shared_act_table():
    """Make the act-table pass resolve Exp and Ln to the one table that
    holds both, so the unrolled loop needs a single table load."""
    import concourse.bacc as bacc_mod
    from concourse.hw_specs import get_activation_tables as orig
    from concourse import mybir

    pref = "natural_log_exp_and_others"
    both = {mybir.ActivationFunctionType.Exp, mybir.ActivationFunctionType.Ln}

    def patched(arch):
        t = orig(arch)
        if pref not in t or not both.issubset(set(t[pref])):
            return t
        # Keep dict order (act_func_set_id is positional); hide Exp/Ln from
        # every other table so the pass resolves both to the shared one.
        return {
            k: v if k == pref else type(v)(f for f in v if f not in both)
            for k, v in t.items()
        }

    bacc_mod.get_activation_tables = patched


def _build_nc(U, p_last, V, num_devices=_NCORES):
    """Build + compile the SPMD Bass program."""
    import concourse.tile as tile
    from concourse import bacc, library_config, mybir

    _prefer_shared_act_table()
    nc = bacc.Bacc(
        "TRN2", target_bir_lowering=False, debug=False, num_devices=num_devices
    )
    f32 = mybir.dt.float32
    f16 = mybir.dt.float16
    u32 = mybir.dt.uint32
    u8 = mybir.dt.uint8
    i16 = mybir.dt.int16
    VX = V + 2  # sentinel column pair appended
    chunks = _chunks(U, p_last)
    NMAX = max(len(c) for c in chunks)

    X = nc.dram_tensor("xin", [U, _P, VX], f16, kind="ExternalInput").ap()
    IDX = nc.dram_tensor("idxin", [_P, U * _IDXW], i16, kind="ExternalInput").ap()
    SCL = nc.dram_tensor("sclin", [_P, U], f32, kind="ExternalInput").ap()
    PAR = nc.dram_tensor("parin", [_P, U * _NIDX], u8, kind="ExternalInput").ap()
    RES = nc.dram_tensor("resout", [_P, U], f32, kind="ExternalOutput").ap()

    sub = mybir.AluOpType.subtract
    mult = mybir.AluOpType.mult
    add = mybir.AluOpType.add
    f_exp = mybir.ActivationFunctionType.Exp
    f_ln = mybir.ActivationFunctionType.Ln

    with tile.TileContext(nc) as tc:
        with (
            tc.tile_pool(name="xp", bufs=2) as xp,
            tc.tile_pool(name="meta", bufs=1) as mp,
            tc.tile_pool(name="work", bufs=2) as wp,
            tc.tile_pool(name="resp", bufs=1) as rp,
        ):
            # ap_gather ucode library: load up front so the ~30us IRAM swap
            # overlaps the first X DMA instead of stalling the first gather.
            nc.gpsimd.load_library(library_config.ap_gather)
            # meta loads ride the ACT HWDGE ring so they never queue behind
            # the big X transfers on the SP ring
            idx_t = mp.tile([_P, U * _IDXW], i16)
            nc.scalar.dma_start(idx_t[:], IDX)
            scl_t = mp.tile([_P, U], f32)
            nc.scalar.dma_start(scl_t[:], SCL)
            par_t = mp.tile([_P, U * _NIDX], u8)
            nc.scalar.dma_start(par_t[:], PAR)
            res_t = rp.tile([_P, U], f32)
            nc.vector.memset(res_t[:], 0.0)

            iw = 0  # running offsets into IDX/PAR (in per-region units)
            for chunk in chunks:
                n = len(chunk)
                u0 = chunk[0]
                p = p_last if (p_last != _P and u0 == U - 1) else _P

                xt = xp.tile([_P, NMAX * VX], f16, tag="x")
                nc.sync.dma_start(
                    xt[:p, : n * VX].rearrange("q (t v) -> q t v", t=n),
                    X[u0 : u0 + n].rearrange("t q v -> q t v")[:p],
                )

                gt = wp.tile([_P, NMAX * 2 * _NIDX], f16, tag="g")
                nc.gpsimd.ap_gather(
                    gt[:p, : 2 * n * _NIDX].bitcast(u32),
                    xt[:p, : n * VX].bitcast(u32),
                    idx_t[:p, iw * _IDXW : (iw + n) * _IDXW],
                    p, n * VX // 2, 1, n * _NIDX,
                )
                # layout per row: n regions x (pos 208 | neg 208) x 2 halves
                g5 = gt[:p, : 2 * n * _NIDX].rearrange(
                    "q (r b j h) -> q r b j h", r=n, b=2, j=_JP
                )
                par3 = par_t[:p, iw * _NIDX : (iw + n) * _NIDX].rearrange(
                    "q (r b j) -> q r b j", r=n, b=2
                )
                pos = wp.tile([_P, NMAX * _JP], f16, tag="pos")
                neg = wp.tile([_P, NMAX * _JP], f16, tag="neg")
                for r in range(n):
                    ps = pos[:p, r * _JP : (r + 1) * _JP]
                    nc.vector.tensor_copy(ps, g5[:, r, 0, :, 0])
                    nc.vector.copy_predicated(ps, par3[:, r, 0, :], g5[:, r, 0, :, 1])
                    ns = neg[:p, r * _JP : (r + 1) * _JP]
                    nc.vector.tensor_copy(ns, g5[:, r, 1, :, 0])
                    nc.vector.copy_predicated(ns, par3[:, r, 1, :], g5[:, r, 1, :, 1])

                # diff = neg - pos
                dt_ = wp.tile([_P, NMAX * _JP], f32, tag="d")
                nc.vector.scalar_tensor_tensor(
                    dt_[:p, : n * _JP], neg[:p, : n * _JP], 1.0,
                    pos[:p, : n * _JP], op0=mult, op1=sub,
                )
                # softplus(d) = ln(exp(d) + 1); d = neg-pos is bounded
                # (~N(0,2), |d| <~ 15) so exp never overflows in f32, and the
                # sentinel pads give exp(-2*_SENT) == 0 -> softplus == 0.
                et = wp.tile([_P, NMAX * _JP], f32, tag="e")
                nc.scalar.activation(et[:p, : n * _JP], dt_[:p, : n * _JP], f_exp)
                st = wp.tile([_P, NMAX * _JP], f32, tag="s")
                nc.scalar.activation(
                    st[:p, : n * _JP], et[:p, : n * _JP], f_ln, bias=1.0
                )
                # per-partition scale (validity * 1/L^2) with fused reduction
                pt = wp.tile([_P, NMAX * _JP], f32, tag="p")
                for r, u in enumerate(chunk):
                    nc.vector.tensor_scalar(
                        pt[:p, r * _JP : (r + 1) * _JP],
                        st[:p, r * _JP : (r + 1) * _JP],
                        scl_t[:p, u : u + 1], None,
                        op0=mult, op1=add,
                        accum_out=res_t[:p, u : u + 1],
                    )
                iw += n

            nc.sync.dma_start(RES, res_t[:])

    nc.compile()
    return nc


def _prep(output, labels, x_lens, neg_ids):
    """Pack valid rows into per-core region tensors + index/scale metadata."""
    B, T, V = output.shape
    lens = np.asarray(x_lens).astype(np.int64)
    labels = np.asarray(labels).astype(np.int64)
    neg = np.asarray(neg_ids).astype(np.int64)[:, :, 0]
    VX = V + 2
    sent_pair = V // 2  # u32-pair index of the sentinel columns

    # Per-sample flat index row [416] (pair units), parity row [416].
    idx_flat = np.zeros((B, _NIDX), np.int16)
    par_flat = np.zeros((B, _NIDX), np.uint8)
    for b in range(B):
        L = int(lens[b])
        pos_c = labels[b, :L]
        neg_c = neg[b, :L]
        f = idx_flat[b]
        f[:] = sent_pair
        f[:L] = (pos_c // 2).astype(np.int16)
        f[_JP : _JP + L] = (neg_c // 2).astype(np.int16)
        par_flat[b, _JP:] = 1  # sentinel: neg pad reads the -big half
        par_flat[b, :L] = (pos_c % 2).astype(np.uint8)
        par_flat[b, _JP : _JP + L] = (neg_c % 2).astype(np.uint8)

    slots = [(b, r) for b in range(B) for r in range(0, int(lens[b]), _SLOT)]
    S = len(slots)
    K = max(1, math.ceil(S / _NCORES))       # slots per core (identical; SPMD)
    U = math.ceil(K / _GROUPS)               # regions per core
    p_last = _SLOT * (K - _GROUPS * (U - 1))  # rows in the last region

    # slot_owner[c, u, g] = sample id or -1
    slot_owner = np.full((_NCORES, U, _GROUPS), -1, np.int64)
    X = np.zeros((_NCORES, U, _P, VX), np.float16)
    X[..., V] = _SENT
    X[..., V + 1] = -_SENT
    SCL = np.zeros((_NCORES, _P, U), np.float32)
    for s, (b, r) in enumerate(slots):
        c, k = divmod(s, K)
        u, g = divmod(k, _GROUPS)
        L = int(lens[b])
        nr = min(_SLOT, L - r)
        p0 = g * _SLOT
        X[c, u, p0 : p0 + nr, :V] = output[b, r : r + nr].astype(np.float16)
        SCL[c, p0 : p0 + nr, u] = 1.0 / (L * L)
        slot_owner[c, u, g] = b

    # idx/par laid out per gather chunk: region r of a chunk gets its pair
    # indices offset by r * VX//2 (the region's position inside the SBUF tile)
    chunks = _chunks(U, p_last)
    IDX = np.zeros((_NCORES, _P, U, _IDXW), np.int16)
    PAR = np.zeros((_NCORES, _P, U, _NIDX), np.uint8)
    for c in range(_NCORES):
        iw = 0
        for chunk in chunks:
            n = len(chunk)
            for g in range(_GROUPS):
                p0 = g * _SLOT
                flat = np.zeros(n * _NIDX, np.int16)
                parf = np.zeros(n * _NIDX, np.uint8)
                for r, u in enumerate(chunk):
                    b = slot_owner[c, u, g]
                    base = r * (VX // 2)
                    if b >= 0:
                        flat[r * _NIDX : (r + 1) * _NIDX] = idx_flat[b] + base
                        parf[r * _NIDX : (r + 1) * _NIDX] = par_flat[b]
                    else:
                        flat[r * _NIDX : (r + 1) * _NIDX] = sent_pair + base
                        parf[r * _NIDX + _JP : (r + 1) * _NIDX] = 1
                wrapped = flat.reshape(n * _IDXW, _SLOT).T      # [16, n*26]
                IDX[c, p0 : p0 + _SLOT, iw : iw + n] = wrapped.reshape(
                    _SLOT, n, _IDXW
                )
                PAR[c, p0 : p0 + _SLOT, iw : iw + n] = parf.reshape(
                    1, n, _NIDX
                )
            iw += n

    return (
        U,
        p_last,
        X,
        IDX.reshape(_NCORES, _P, U * _IDXW),
        SCL,
        PAR.reshape(_NCORES, _P, U * _NIDX),
    )


def _run(inputs, trace=False, tmpdir=None, trace_cores=None):
    from concourse import bass_utils

    output = np.asarray(inputs["output"], np.float32)
    U, p_last, X, IDX, SCL, PAR = _prep(
        output, inputs["labels"], inputs["x_lens"], inputs["neg_ids"]
    )
    key = (U, p_last, output.shape[2])
    if key not in _nc_cache:
        _nc_cache[key] = _build_nc(U, p_last, output.shape[2])
    nc = _nc_cache[key]

    in_maps = [
        {"xin": X[c], "idxin": IDX[c], "sclin": SCL[c], "parin": PAR[c]}
        for c in range(_NCORES)
    ]
    br = bass_utils.run_bass_kernel_spmd(
        nc, in_maps, core_ids=list(range(_NCORES)), trace=trace, tmpdir=tmpdir,
        trace_cores=trace_cores,
    )
    total = np.float64(0.0)
    for c in range(_NCORES):
        total += np.asarray(br.results[c]["resout"], np.float64).sum()
    loss = np.array([total], np.float32)
    return loss, br


def kernel(**inputs) -> np.ndarray:
    loss, _ = _run(inputs, trace=False)
    return loss
